# revision 8
# baseline (speedup 1.0000x reference)
"""Cosine-similarity loss kernel for Trainium2 (8 NeuronCores, SPMD).

loss = -sum_n dot(s_n, im_n) / (||s_n|| * ||im_n||)   for s, im in R^{65536 x 512}

Strategy (memory-bound, ~360-380 GB/s HBM per core):
  - Shard the 65536 rows across 8 cores (8192 rows each, 32 MB/core streamed
    via HWDGE on nc.sync; one queue spread over 16 DMA engines saturates HBM).
  - Per 128-row slice (64 slices/core), three fused one-pass reductions:
      dot = sum_d s*im  -> VectorE scalar_tensor_tensor (s*1)*im, accum_out
      ss  = sum_d s*s   -> ScalarE activation(Square, accum_out)
      ii  = sum_d im*im -> split DVE/ACT (20/64 on ACT) to balance engines;
            per-engine ii tiles (zeroed once on GpSimd) avoid cross-engine
            WAW semaphore ping-pong on a shared accumulator tile.
  - Single ACT table set: a warmup Abs_reciprocal_sqrt pins set 15 (which
    also contains Square) at startup -> no mid-kernel/tail table switches.
  - Tail: loss_p[128,1] = -sum_c dot_c * abs_rsqrt(ss_c*ii_c); PE matmul
    against a ones vector folds the 128 partials into one PSUM scalar, so
    the output DMA is a single 4B descriptor (a [128,1] DMA costs 16
    serialized per-DMA-engine completion-semaphore posts, ~6us of tail).
  - Host sums the 8 per-core scalars -> f32.
"""

import numpy as np
from contextlib import ExitStack

import concourse.bacc as bacc
import concourse.bass as bass
import concourse.mybir as mybir
import concourse.tile as tile
from concourse.bass_utils import run_bass_kernel_spmd

N, D = 65536, 512
N_CORES = 8
ROWS = N // N_CORES          # 8192 rows per core
P = 128                      # SBUF partitions
F32 = mybir.dt.float32
BF16 = mybir.dt.bfloat16


def _build(
    rows=ROWS,
    # slices per DMA tile (1 slice = 128 rows = 256KB fp32 / tensor).  Small
    # first tiles start compute early; small last tiles shrink the post-DMA
    # tail; few large middle tiles keep the semaphore count (and the
    # epilogue's per-semaphore reset chatter) low.
    seg_schedule=(1, 1, 2, 4) + (8,) * 6 + (4, 2, 1, 1),
    bufs=5,
    dtype="f32",             # SBUF tile dtype; "bf16" casts in the DMA
    # which slices' ii goes to ACT (~20/64), none in the last 8 so the
    # post-DMA tail drains on both engines instead of serializing on ACT
    ii_on_act=lambda c: c % 14 >= 9 and c < 56,
    mapping="pj",            # row->partition: pj = p*seg+j gives contiguous
                             # per-partition DMA segments
):
    slices = rows // P
    assert sum(seg_schedule) == slices
    DT = BF16 if dtype == "bf16" else F32

    nc = bacc.Bacc(
        "TRN2", target_bir_lowering=False, debug=False, num_devices=N_CORES
    )
    s_d = nc.dram_tensor("s", [rows, D], F32, kind="ExternalInput").ap()
    im_d = nc.dram_tensor("im", [rows, D], F32, kind="ExternalInput").ap()
    out_d = nc.dram_tensor("out", [1, 1], F32, kind="ExternalOutput").ap()

    mult = mybir.AluOpType.mult
    add = mybir.AluOpType.add

    with tile.TileContext(nc) as tc, ExitStack() as ctx:
        spool = ctx.enter_context(tc.tile_pool(name="spool", bufs=bufs))
        ipool = ctx.enter_context(tc.tile_pool(name="ipool", bufs=bufs))
        stats = ctx.enter_context(tc.tile_pool(name="stats", bufs=1))
        ppool = ctx.enter_context(
            tc.tile_pool(name="ppool", bufs=1, space="PSUM")
        )

        dot_all = stats.tile([P, slices], F32)
        ss_all = stats.tile([P, slices], F32)
        ii_dve = stats.tile([P, slices], F32)
        ii_act = stats.tile([P, slices], F32)
        dve_scr = stats.tile([P, D], DT)
        act_scr = stats.tile([P, D], DT)
        warm = stats.tile([P, 1], F32)   # ones: ACT warmup + final PE reduce

        nc.gpsimd.memset(ii_dve[:], 0.0)
        nc.gpsimd.memset(ii_act[:], 0.0)
        nc.gpsimd.memset(warm[:], 1.0)

        # Pin the ACT table to set 15 (abs_reciprocal_sqrt + square) before
        # the first Square, so the whole kernel needs exactly one table load,
        # issued during DMA warmup instead of at the tail.
        nc.scalar.activation(
            act_scr[:, 0:1], warm[:],
            mybir.ActivationFunctionType.Abs_reciprocal_sqrt,
        )

        c = 0
        r0 = 0
        pat = "(j p) d -> p j d" if mapping == "jp" else "(p j) d -> p j d"
        dma_eng = nc.gpsimd if dtype == "bf16" else nc.sync
        for seg in seg_schedule:
            nrows = seg * P
            s_seg = s_d[r0 : r0 + nrows, :].rearrange(pat, p=P, j=seg)
            im_seg = im_d[r0 : r0 + nrows, :].rearrange(pat, p=P, j=seg)
            r0 += nrows
            st = spool.tile([P, seg, D], DT, name="st", tag="st")
            dma_eng.dma_start(st[:], s_seg)
            it = ipool.tile([P, seg, D], DT, name="it", tag="it")
            dma_eng.dma_start(it[:], im_seg)
            for j in range(seg):
                nc.vector.scalar_tensor_tensor(
                    out=dve_scr[:], in0=st[:, j, :], scalar=1.0, in1=it[:, j, :],
                    op0=mult, op1=mult,
                    accum_out=dot_all[:, c : c + 1],
                )
                nc.scalar.activation(
                    out=act_scr[:], in_=st[:, j, :],
                    func=mybir.ActivationFunctionType.Square,
                    accum_out=ss_all[:, c : c + 1],
                )
                if ii_on_act(c):
                    nc.scalar.activation(
                        out=act_scr[:], in_=it[:, j, :],
                        func=mybir.ActivationFunctionType.Square,
                        accum_out=ii_act[:, c : c + 1],
                    )
                else:
                    nc.vector.scalar_tensor_tensor(
                        out=dve_scr[:], in0=it[:, j, :], scalar=1.0, in1=it[:, j, :],
                        op0=mult, op1=mult,
                        accum_out=ii_dve[:, c : c + 1],
                    )
                c += 1

        # tail: loss = -sum_c dot_c * abs_rsqrt(ss_c*ii_c), folded to a
        # single PSUM scalar via PE so the out-DMA is one descriptor.
        ii_sum = stats.tile([P, slices], F32)
        nc.vector.tensor_tensor(out=ii_sum[:], in0=ii_dve[:], in1=ii_act[:], op=add)
        prod = stats.tile([P, slices], F32)
        nc.vector.tensor_tensor(out=prod[:], in0=ss_all[:], in1=ii_sum[:], op=mult)
        rt = stats.tile([P, slices], F32)
        nc.scalar.activation(
            rt[:], prod[:], mybir.ActivationFunctionType.Abs_reciprocal_sqrt
        )
        fin_scr = stats.tile([P, slices], F32)
        loss_p = stats.tile([P, 1], F32)
        nc.vector.scalar_tensor_tensor(
            out=fin_scr[:], in0=dot_all[:], scalar=-1.0, in1=rt[:],
            op0=mult, op1=mult,
            accum_out=loss_p[:],
        )
        ps = ppool.tile([1, 1], F32)
        nc.tensor.matmul(ps[:], loss_p[:], warm[:], start=True, stop=True)
        loss_s = stats.tile([1, 1], F32)
        nc.vector.tensor_copy(out=loss_s[:], in_=ps[:])
        nc.sync.dma_start(out_d, loss_s[:])

    nc.compile()
    return nc


_compiled = None


def _get_nc():
    global _compiled
    if _compiled is None:
        _compiled = _build()
    return _compiled


def _run(s, im, nc=None, **kw):
    """Shard, run on 8 cores, return BassKernelResults."""
    s = np.ascontiguousarray(np.asarray(s, dtype=np.float32))
    im = np.ascontiguousarray(np.asarray(im, dtype=np.float32))
    assert s.shape == (N, D) and im.shape == (N, D)
    if nc is None:
        nc = _get_nc()
    in_maps = [
        {"s": s[c * ROWS : (c + 1) * ROWS], "im": im[c * ROWS : (c + 1) * ROWS]}
        for c in range(N_CORES)
    ]
    bkr = run_bass_kernel_spmd(nc, in_maps, core_ids=list(range(N_CORES)), **kw)
    return bkr


def kernel(s, im, temp=None, **_):
    bkr = _run(s, im)
    total = np.float64(0.0)
    for r in bkr.results:
        total += r["out"].astype(np.float64).sum()
    return np.float32(total)


# revision 10
# speedup vs baseline: 1.1621x; 1.1621x over previous
"""Cosine-similarity loss kernel for Trainium2 (8 NeuronCores, SPMD).

loss = -sum_n dot(s_n, im_n) / (||s_n|| * ||im_n||)   for s, im in R^{65536 x 512}

Strategy (memory-bound, ~360-380 GB/s HBM per core):
  - Shard the 65536 rows across 8 cores (8192 rows each, 32 MB/core streamed
    via HWDGE on nc.sync; one queue spread over 16 DMA engines saturates HBM).
  - Per 128-row slice (64 slices/core), three fused one-pass reductions:
      dot = sum_d s*im  -> VectorE scalar_tensor_tensor (s*1)*im, accum_out
      ss  = sum_d s*s   -> ScalarE activation(Square, accum_out)
      ii  = sum_d im*im -> split DVE/ACT (20/64 on ACT) to balance engines;
            per-engine ii tiles (zeroed once on GpSimd) avoid cross-engine
            WAW semaphore ping-pong on a shared accumulator tile.
  - Single ACT table set: a warmup Abs_reciprocal_sqrt pins set 15 (which
    also contains Square) at startup -> no mid-kernel/tail table switches.
  - Tail: loss_p[128,1] = -sum_c dot_c * abs_rsqrt(ss_c*ii_c); PE matmul
    against a ones vector folds the 128 partials into one PSUM scalar, so
    the output DMA is a single 4B descriptor (a [128,1] DMA costs 16
    serialized per-DMA-engine completion-semaphore posts, ~6us of tail).
  - Host sums the 8 per-core scalars -> f32.
"""

import numpy as np
from contextlib import ExitStack

import concourse.bacc as bacc
import concourse.bass as bass
import concourse.mybir as mybir
import concourse.tile as tile
from concourse.bass_utils import run_bass_kernel_spmd

N, D = 65536, 512
N_CORES = 8
ROWS = N // N_CORES          # 8192 rows per core
P = 128                      # SBUF partitions
F32 = mybir.dt.float32
BF16 = mybir.dt.bfloat16


def _build(
    rows=ROWS,
    # slices per DMA tile (1 slice = 128 rows = 256KB fp32 / tensor).  Small
    # first tiles start compute early; small last tiles shrink the post-DMA
    # tail; few large middle tiles keep the semaphore count (and the
    # epilogue's per-semaphore reset chatter) low.
    seg_schedule=(1, 1, 2) + (4,) * 14 + (2, 1, 1),
    bufs=10,
    dtype="f32",             # SBUF tile dtype; "bf16" casts in the DMA
    # which slices' ii goes to ACT (~20/64), none in the last 8 so the
    # post-DMA tail drains on both engines instead of serializing on ACT
    ii_on_act=lambda c: c % 3 == 2 and c < 60,
    mapping="pj",            # row->partition: pj = p*seg+j gives contiguous
                             # per-partition DMA segments
):
    slices = rows // P
    assert sum(seg_schedule) == slices
    DT = BF16 if dtype == "bf16" else F32

    nc = bacc.Bacc(
        "TRN2", target_bir_lowering=False, debug=False, num_devices=N_CORES
    )
    s_d = nc.dram_tensor("s", [rows, D], F32, kind="ExternalInput").ap()
    im_d = nc.dram_tensor("im", [rows, D], F32, kind="ExternalInput").ap()
    out_d = nc.dram_tensor("out", [1, 1], F32, kind="ExternalOutput").ap()

    mult = mybir.AluOpType.mult
    add = mybir.AluOpType.add

    with tile.TileContext(nc) as tc, ExitStack() as ctx:
        spool = ctx.enter_context(tc.tile_pool(name="spool", bufs=bufs))
        ipool = ctx.enter_context(tc.tile_pool(name="ipool", bufs=bufs))
        stats = ctx.enter_context(tc.tile_pool(name="stats", bufs=1))
        ppool = ctx.enter_context(
            tc.tile_pool(name="ppool", bufs=1, space="PSUM")
        )

        dot_all = stats.tile([P, slices], F32)
        ss_all = stats.tile([P, slices], F32)
        ii_dve = stats.tile([P, slices], F32)
        ii_act = stats.tile([P, slices], F32)
        dve_scr = stats.tile([P, D], DT)
        act_scr = stats.tile([P, D], DT)
        warm = stats.tile([P, 1], F32)   # ones: ACT warmup + final PE reduce

        nc.gpsimd.memset(ii_dve[:], 0.0)
        nc.gpsimd.memset(ii_act[:], 0.0)
        nc.gpsimd.memset(warm[:], 1.0)

        # Pin the ACT table to set 15 (abs_reciprocal_sqrt + square) before
        # the first Square, so the whole kernel needs exactly one table load,
        # issued during DMA warmup instead of at the tail.
        nc.scalar.activation(
            act_scr[:, 0:1], warm[:],
            mybir.ActivationFunctionType.Abs_reciprocal_sqrt,
        )

        c = 0
        r0 = 0
        pat = "(j p) d -> p j d" if mapping == "jp" else "(p j) d -> p j d"
        dma_eng = nc.gpsimd if dtype == "bf16" else nc.sync
        for seg in seg_schedule:
            nrows = seg * P
            s_seg = s_d[r0 : r0 + nrows, :].rearrange(pat, p=P, j=seg)
            im_seg = im_d[r0 : r0 + nrows, :].rearrange(pat, p=P, j=seg)
            r0 += nrows
            st = spool.tile([P, seg, D], DT, name="st", tag="st")
            dma_eng.dma_start(st[:], s_seg)
            it = ipool.tile([P, seg, D], DT, name="it", tag="it")
            dma_eng.dma_start(it[:], im_seg)
            for j in range(seg):
                nc.vector.scalar_tensor_tensor(
                    out=dve_scr[:], in0=st[:, j, :], scalar=1.0, in1=it[:, j, :],
                    op0=mult, op1=mult,
                    accum_out=dot_all[:, c : c + 1],
                )
                nc.scalar.activation(
                    out=act_scr[:], in_=st[:, j, :],
                    func=mybir.ActivationFunctionType.Square,
                    accum_out=ss_all[:, c : c + 1],
                )
                if ii_on_act(c):
                    nc.scalar.activation(
                        out=act_scr[:], in_=it[:, j, :],
                        func=mybir.ActivationFunctionType.Square,
                        accum_out=ii_act[:, c : c + 1],
                    )
                else:
                    nc.vector.scalar_tensor_tensor(
                        out=dve_scr[:], in0=it[:, j, :], scalar=1.0, in1=it[:, j, :],
                        op0=mult, op1=mult,
                        accum_out=ii_dve[:, c : c + 1],
                    )
                c += 1

        # tail: loss = -sum_c dot_c * abs_rsqrt(ss_c*ii_c), folded to a
        # single PSUM scalar via PE so the out-DMA is one descriptor.
        ii_sum = stats.tile([P, slices], F32)
        nc.vector.tensor_tensor(out=ii_sum[:], in0=ii_dve[:], in1=ii_act[:], op=add)
        prod = stats.tile([P, slices], F32)
        nc.vector.tensor_tensor(out=prod[:], in0=ss_all[:], in1=ii_sum[:], op=mult)
        rt = stats.tile([P, slices], F32)
        nc.scalar.activation(
            rt[:], prod[:], mybir.ActivationFunctionType.Abs_reciprocal_sqrt
        )
        fin_scr = stats.tile([P, slices], F32)
        loss_p = stats.tile([P, 1], F32)
        nc.vector.scalar_tensor_tensor(
            out=fin_scr[:], in0=dot_all[:], scalar=-1.0, in1=rt[:],
            op0=mult, op1=mult,
            accum_out=loss_p[:],
        )
        ps = ppool.tile([1, 1], F32)
        nc.tensor.matmul(ps[:], loss_p[:], warm[:], start=True, stop=True)
        loss_s = stats.tile([1, 1], F32)
        nc.vector.tensor_copy(out=loss_s[:], in_=ps[:])
        nc.sync.dma_start(out_d, loss_s[:])

    nc.compile()
    return nc


_compiled = None


def _get_nc():
    global _compiled
    if _compiled is None:
        _compiled = _build()
    return _compiled


def _run(s, im, nc=None, **kw):
    """Shard, run on 8 cores, return BassKernelResults."""
    s = np.ascontiguousarray(np.asarray(s, dtype=np.float32))
    im = np.ascontiguousarray(np.asarray(im, dtype=np.float32))
    assert s.shape == (N, D) and im.shape == (N, D)
    if nc is None:
        nc = _get_nc()
    in_maps = [
        {"s": s[c * ROWS : (c + 1) * ROWS], "im": im[c * ROWS : (c + 1) * ROWS]}
        for c in range(N_CORES)
    ]
    bkr = run_bass_kernel_spmd(nc, in_maps, core_ids=list(range(N_CORES)), **kw)
    return bkr


def kernel(s, im, temp=None, **_):
    bkr = _run(s, im)
    total = np.float64(0.0)
    for r in bkr.results:
        total += r["out"].astype(np.float64).sum()
    return np.float32(total)
